# revision 9
# baseline (speedup 1.0000x reference)
"""Batched TGCN (GCN-GRU) Trainium2 kernel.

Strategy:
  - GCNConv is linear in x: segment_sum(norm * (X W)[src] -> dst) == (A_norm @ X) W.
    So the graph aggregation A_norm @ X is done ONCE over all B*Fin*T = 96 feature
    columns, shared by all 3 gates and all 12 timesteps.
  - Host: builds the dense normalized adjacency (incl. self loops) in bf16, folds
    the GCN weight into the GRU input weights, reorders x.
  - Device (8 cores, node-sharded 1250 dst nodes/core, zero cross-core comms):
      Stage 1: aggT[96, 1250] = sum_k X_chunk[k].T @ A_T_chunk[k]  (bf16 matmuls,
               fp32 PSUM accumulation), then de-interleave batch into columns.
      Stage 2: 12-step GRU recurrence in feature-major layout [64, 5000]
               (columns = local_node*4 + batch), all state SBUF-resident.
  - Output: [12, 16, 5000] fp32 per core, host reassembles [B, T, N, OUT].
"""

import numpy as np
import ml_dtypes

import concourse.bass as bass
import concourse.mybir as mybir
import concourse.tile as tile
from concourse import bacc
from concourse.bass import ds, ts

F32 = mybir.dt.float32
BF16 = mybir.dt.bfloat16
AF = mybir.ActivationFunctionType

# Problem constants (hardcoded per contract)
N_NODES = 10000
N_EDGES = 320000
FIN = 2
HID = 64
OUT = 16
B = 4
T = 12
N_CORES = 8

P = 128
NPC = N_NODES // N_CORES          # 1250 dst nodes per core
FCOL = NPC * B                    # 5000 recurrence columns per core
NFEAT = B * FIN * T               # 96 aggregation features
N_SRC_PAD = ((N_NODES + P - 1) // P) * P   # 10112
KCH = N_SRC_PAD // P              # 79 k-chunks
KB = 4                            # k-chunks per A-stream DMA
DST_TILE = 512                    # aggregation psum tile width
CHUNK = 1000                      # recurrence column chunk (5 chunks of 1000)


def build_program(npc=NPC, kch=KCH, chunk=CHUNK):
    """Build the per-core Bass program. All 8 cores run the same program."""
    fcol = npc * B
    n_chunks = (fcol + chunk - 1) // chunk
    dst_tiles = []
    o = 0
    while o < npc:
        w = min(DST_TILE, npc - o)
        dst_tiles.append((o, w))
        o += w

    nc = bacc.Bacc("TRN2", target_bir_lowering=False, debug=False)

    def mm_tiled(out, lhsT, rhs, start, stop):
        # matmul free dim must fit one PSUM bank (<=512 fp32 columns)
        w = out.shape[-1]
        for o in range(0, w, 512):
            ww = min(512, w - o)
            nc.tensor.matmul(out[:, o : o + ww], lhsT=lhsT, rhs=rhs[:, o : o + ww],
                             start=start, stop=stop)

    a_t = nc.dram_tensor("a_t", [P, kch, npc], BF16, kind="ExternalInput")
    x_feat = nc.dram_tensor("x_feat", [P, kch, NFEAT], BF16, kind="ExternalInput")
    w_zr = nc.dram_tensor("w_zr", [HID + FIN, 2 * HID], BF16, kind="ExternalInput")
    w_h = nc.dram_tensor("w_h", [HID + FIN, HID], BF16, kind="ExternalInput")
    w_lin = nc.dram_tensor("w_lin", [HID, OUT], BF16, kind="ExternalInput")
    b_zr = nc.dram_tensor("b_zr", [2 * HID, 1], F32, kind="ExternalInput")
    b_h = nc.dram_tensor("b_h", [HID, 1], F32, kind="ExternalInput")
    b_lin = nc.dram_tensor("b_lin", [OUT, 1], F32, kind="ExternalInput")
    out_d = nc.dram_tensor("out", [T, OUT, fcol], F32, kind="ExternalOutput")

    with tile.TileContext(nc) as tc:
        with tc.tile_pool(name="persist", bufs=1) as pp:
            # persistent SBUF tensors
            x_sb = pp.tile([P, kch, NFEAT], BF16, tag="x_sb")
            nc.sync.dma_start(x_sb[:], x_feat[:])

            wzr = pp.tile([HID + FIN, 2 * HID], BF16, tag="wzr")
            nc.sync.dma_start(wzr[:], w_zr[:])
            wh = pp.tile([HID + FIN, HID], BF16, tag="wh")
            nc.sync.dma_start(wh[:], w_h[:])
            wlin = pp.tile([HID, OUT], BF16, tag="wlin")
            nc.sync.dma_start(wlin[:], w_lin[:])
            bz_t = pp.tile([HID, 1], F32, tag="bz_t")
            nc.sync.dma_start(bz_t[:], b_zr[:HID])
            br_t = pp.tile([HID, 1], F32, tag="br_t")
            nc.sync.dma_start(br_t[:], b_zr[HID:])
            bh = pp.tile([HID, 1], F32, tag="bh")
            nc.sync.dma_start(bh[:], b_h[:])
            blin = pp.tile([OUT, 1], F32, tag="blin")
            nc.sync.dma_start(blin[:], b_lin[:])

            agg_nodes = pp.tile([NFEAT, npc], BF16, tag="agg_nodes")
            aggT = pp.tile([FIN * T, fcol], BF16, tag="aggT")
            h_bufs = [pp.tile([HID + FIN, fcol], BF16, tag=f"h{i}", name=f"h{i}") for i in range(2)]
            rh_bufs = [pp.tile([HID + FIN, fcol], BF16, tag=f"rh{i}", name=f"rh{i}") for i in range(2)]
            nc.gpsimd.memset(h_bufs[0][:HID, :], 0.0)
            out_sb = [pp.tile([OUT, fcol], F32, tag=f"osb{i}", name=f"osb{i}") for i in range(2)]

            # ---------------- Stage 1: aggregation ----------------
            with (
                tc.tile_pool(name="astream", bufs=2) as ap_,
                tc.tile_pool(name="apsum", bufs=1, space="PSUM") as aps,
            ):
                psums = [aps.tile([NFEAT, w], F32, tag=f"agp{i}", name=f"agp{i}")
                         for i, (_, w) in enumerate(dst_tiles)]
                n_ktiles = (kch + KB - 1) // KB
                for kt in range(n_ktiles):
                    k0 = kt * KB
                    kb = min(KB, kch - k0)
                    a_sb = ap_.tile([P, KB, npc], BF16, tag="a_sb")
                    nc.sync.dma_start(a_sb[:, :kb], a_t[:, k0 : k0 + kb, :])
                    for kl in range(kb):
                        k = k0 + kl
                        for i, (doff, w) in enumerate(dst_tiles):
                            nc.tensor.matmul(
                                psums[i][:],
                                lhsT=x_sb[:, k, :],
                                rhs=a_sb[:, kl, ds(doff, w)],
                                start=(k == 0),
                                stop=(k == kch - 1),
                            )
                for i, (doff, w) in enumerate(dst_tiles):
                    nc.vector.tensor_copy(agg_nodes[:, ds(doff, w)], psums[i][:])

            # de-interleave batch (features are b-major blocks of FIN*T=24 rows):
            # aggT[t*2+fin, n*B+b] = agg_nodes[b*24 + t*2 + fin, n]
            for b in range(B):
                nc.sync.dma_start(
                    aggT[:, b::B],
                    agg_nodes[b * (FIN * T) : (b + 1) * (FIN * T), :],
                )

            # ---------------- Stage 2: GRU recurrence ----------------
            with (
                tc.tile_pool(name="work", bufs=3) as wp,
                tc.tile_pool(name="pzr", bufs=2, space="PSUM") as pzr_pool,
                tc.tile_pool(name="ph", bufs=1, space="PSUM") as ph_pool,
                tc.tile_pool(name="pl", bufs=1, space="PSUM") as pl_pool,
            ):
                for t in range(T):
                    h_prev = h_bufs[t % 2]
                    h_next = h_bufs[(t + 1) % 2]
                    rh_full = rh_bufs[t % 2]
                    ot_sb = out_sb[t % 2]
                    # per-t aggregated input rows into the matmul rhs tensors
                    nc.sync.dma_start(h_prev[HID:, :], aggT[2 * t : 2 * t + 2, :])
                    nc.sync.dma_start(rh_full[HID:, :], aggT[2 * t : 2 * t + 2, :])
                    for c in range(n_chunks):
                        cw = min(chunk, fcol - c * chunk)
                        cc = ds(c * chunk, cw)

                        # ZR gate pre-activation [128, cw]
                        pz = pzr_pool.tile([2 * HID, chunk], F32, tag="pz", name="pz")[:, :cw]
                        mm_tiled(pz, wzr[:], h_prev[:, cc], True, True)
                        zs = wp.tile([HID, chunk], BF16, tag="zs", name="zs")[:, :cw]
                        nc.scalar.activation(zs, pz[:HID, :], AF.Sigmoid, bias=bz_t[:, 0:1])
                        rs = wp.tile([HID, chunk], BF16, tag="rs", name="rs")[:, :cw]
                        nc.scalar.activation(rs, pz[HID:, :], AF.Sigmoid, bias=br_t[:, 0:1])

                        # H~ pre-activation [64, cw]
                        nc.vector.tensor_tensor(
                            rh_full[:HID, cc], in0=rs, in1=h_prev[:HID, cc],
                            op=mybir.AluOpType.mult)
                        ph = ph_pool.tile([HID, chunk], F32, tag="ph", name="ph")[:, :cw]
                        mm_tiled(ph, wh[:], rh_full[:, cc], True, True)
                        ht = wp.tile([HID, chunk], BF16, tag="ht", name="ht")[:, :cw]
                        nc.scalar.activation(ht, ph, AF.Tanh, bias=bh[:, 0:1])

                        # Hn = ht + Z*(h_prev - ht)
                        d0 = wp.tile([HID, chunk], BF16, tag="d0", name="d0")[:, :cw]
                        nc.vector.tensor_tensor(d0, in0=h_prev[:HID, cc], in1=ht,
                                                op=mybir.AluOpType.subtract)
                        d1 = wp.tile([HID, chunk], BF16, tag="d1", name="d1")[:, :cw]
                        nc.vector.tensor_tensor(d1, in0=zs, in1=d0,
                                                op=mybir.AluOpType.mult)
                        nc.vector.tensor_tensor(h_next[:HID, cc], in0=ht, in1=d1,
                                                op=mybir.AluOpType.add)

                        # out_t = relu(Hn) @ w_lin + b_lin
                        rl = wp.tile([HID, chunk], BF16, tag="rl", name="rl")[:, :cw]
                        nc.vector.tensor_scalar_max(rl, h_next[:HID, cc], 0.0)
                        plin = pl_pool.tile([OUT, chunk], F32, tag="plin", name="plin")[:, :cw]
                        mm_tiled(plin, wlin[:], rl, True, True)
                        nc.scalar.activation(ot_sb[:, cc], plin, AF.Identity,
                                             bias=blin[:, 0:1])
                    nc.sync.dma_start(out_d[t], ot_sb[:])

    nc.compile()
    return nc


def _prep_host(x, edge_index, edge_weight, Wz, bz, Wr, br, Wh, bh,
               Lz_w, Lz_b, Lr_w, Lr_b, Lh_w, Lh_b, lin_w, lin_b,
               n_nodes=N_NODES, npc=NPC, n_cores=N_CORES):
    """Host-side preprocessing: norm, dense A, feature reorder, weight folding."""
    bf16 = ml_dtypes.bfloat16
    n_src_pad = ((n_nodes + P - 1) // P) * P
    kch = n_src_pad // P

    src = np.asarray(edge_index[0], dtype=np.int64)
    dst = np.asarray(edge_index[1], dtype=np.int64)
    w = np.asarray(edge_weight, dtype=np.float32)
    loop = np.arange(n_nodes, dtype=np.int64)
    src_a = np.concatenate([src, loop])
    dst_a = np.concatenate([dst, loop])
    w_a = np.concatenate([w, np.ones(n_nodes, np.float32)])
    deg = np.zeros(n_nodes, np.float32)
    np.add.at(deg, dst_a, w_a)
    dinv = np.where(deg > 0, 1.0 / np.sqrt(deg), 0.0).astype(np.float32)
    norm = dinv[src_a] * w_a * dinv[dst_a]

    A = np.zeros((n_src_pad, n_nodes), np.float32)   # A[src, dst]
    np.add.at(A, (src_a, dst_a), norm)
    A_bf = A.astype(bf16)

    # X rows: [n_src_pad, 96], f = b*24 + t*2 + fin  (b outer, fin inner)
    X = np.zeros((n_src_pad, B * T * FIN), np.float32)
    X[:n_nodes] = np.transpose(np.asarray(x, np.float32), (1, 0, 3, 2)).reshape(n_nodes, -1)
    x_feat = np.ascontiguousarray(
        X.astype(bf16).reshape(kch, P, B * T * FIN).transpose(1, 0, 2))

    f32 = np.float32
    Wz, Wr, Wh = np.asarray(Wz, f32), np.asarray(Wr, f32), np.asarray(Wh, f32)
    Lz_w, Lr_w, Lh_w = np.asarray(Lz_w, f32), np.asarray(Lr_w, f32), np.asarray(Lh_w, f32)
    Az, Ar, Ah = Wz @ Lz_w[:HID], Wr @ Lr_w[:HID], Wh @ Lh_w[:HID]
    Lz2, Lr2, Lh2 = Lz_w[HID:], Lr_w[HID:], Lh_w[HID:]
    bz_f = np.asarray(bz, f32) @ Lz_w[:HID] + np.asarray(Lz_b, f32)
    br_f = np.asarray(br, f32) @ Lr_w[:HID] + np.asarray(Lr_b, f32)
    bh_f = np.asarray(bh, f32) @ Lh_w[:HID] + np.asarray(Lh_b, f32)

    w_zr_np = np.concatenate([np.concatenate([Lz2, Lr2], axis=1),
                              np.concatenate([Az, Ar], axis=1)], axis=0)
    w_h_np = np.concatenate([Lh2, Ah], axis=0)
    common = {
        "x_feat": x_feat,
        "w_zr": np.ascontiguousarray(w_zr_np).astype(bf16),
        "w_h": np.ascontiguousarray(w_h_np).astype(bf16),
        "w_lin": np.asarray(lin_w, f32).astype(bf16),
        "b_zr": np.concatenate([bz_f, br_f]).astype(f32).reshape(-1, 1),
        "b_h": bh_f.astype(f32).reshape(-1, 1),
        "b_lin": np.asarray(lin_b, f32).reshape(-1, 1),
    }
    in_maps = []
    for c in range(n_cores):
        a_core = np.ascontiguousarray(
            A_bf[:, c * npc : (c + 1) * npc].reshape(kch, P, npc).transpose(1, 0, 2))
        in_maps.append(dict(common, a_t=a_core))
    return in_maps


_CACHED_NC = None


def kernel(**inputs) -> np.ndarray:
    global _CACHED_NC
    from concourse.bass_utils import run_bass_kernel_spmd

    in_maps = _prep_host(**inputs)
    if _CACHED_NC is None:
        _CACHED_NC = build_program()
    res = run_bass_kernel_spmd(_CACHED_NC, in_maps, core_ids=list(range(N_CORES)))

    full = np.empty((B, T, N_NODES, OUT), np.float32)
    for c, r in enumerate(res.results):
        o = r["out"].reshape(T, OUT, NPC, B)          # [t, o, n_local, b]
        full[:, :, c * NPC : (c + 1) * NPC, :] = o.transpose(3, 0, 2, 1)
    return full
